# revision 15
# baseline (speedup 1.0000x reference)
"""Trainium2 Bass kernel for DiscreteTimeS4.

Reference computation (per batch element b):
    x_proj = relu(x @ Wi^T + bi)          [T, P]
    u      = x_proj @ B                   [T, H]
    h_t    = a * h_{t-1} + u_t            (diagonal linear scan over T)
    y      = hs @ C                       [T, P]
    out    = y @ Wo^T + bo                [T, O]

Sharding: data-parallel over the batch axis -- core b handles x_seq[b].
Weights replicated. No cross-device communication.

Device strategy (per core):
  - host pre-transposes x to x^T [D, T] so the contraction dim (D) sits on
    SBUF partitions; host fuses W2 = C @ Wo^T so the last two matmuls
    collapse into one; host applies the +bo bias (exact in fp32) so the
    device ships biasless fp16 output (half the store traffic).
  - matmuls run in fp16 (fp32 PSUM accumulation, FWL weight loads); the
    scan runs in exact fp32.
  - startup: DMA descriptors have ~1.3-2us serial per-ring latency, so the
    critical path uses FEW, FUSED descriptors on parallel rings: sync ring
    carries x chunk 0 (one descriptor -- chunk 0 is narrow) then the deep
    x prefetch; the scalar ring carries ALL MM1 weights + the fp32
    bias/decay pack bit-packed into trailing fp16 columns (descriptor 1)
    and the fused B|W2 pack (descriptor 2).  A short stream of dummy
    warmup matmuls runs during the DMA fill to trip the PE HAM clock gate
    (cold 1.2GHz -> warm 2.4GHz) before real work.
  - pipeline over time chunks, tapered on both ends (small first chunk =
    small critical-path DMA; small last chunks = short post-scan tail):
      MM1: XP^T[p,t] = Wi @ x^T          (lhsT = Wi^T, K=512)
      ACT: relu(psum + bi) -> SBUF
      MM2: U^T[h,t] = B^T @ XP^T         (lhsT = B, K=256)
      DVE: tensor_tensor_scan: h = a*h + u along t (fp32 state, carry
           chained across chunks via initial=prev[:, -1:], decay broadcast
           via a zero-stride AP)
      MM3: OUT[t,o] = (HS^T)^T @ W2      (lhsT = HS^T tile -> natural [t,o]
           output layout)
      psum->fp16 evacuation alternates DVE/ACT per 128-row subtile so
      neither engine saturates (GPSIMD has no PSUM port), then one DMA per
      chunk: early chunks on the SWDGE ring, late chunks on sync HWDGE.
"""

import numpy as np

try:
    import concourse.bass as bass
except ImportError:  # pragma: no cover
    import sys

    sys.path.insert(0, "/opt/trn_rl_repo")
    import concourse.bass as bass

from contextlib import ExitStack

import concourse.mybir as mybir
import concourse.tile as tile
from concourse import bacc
from concourse.bass import ts
from concourse.bass_utils import run_bass_kernel_spmd

BSZ, T, D, P, H, O = 8, 4096, 512, 256, 256, 512
F32 = mybir.dt.float32
F16 = mybir.dt.float16

KD = D // 128  # 4 k-tiles for MM1
KP = P // 128  # 2
KH = H // 128  # 2

WIA_F = 2 * P  # fp16 cols of packed Wi^T k-tiles 0,1
MP_F = 2 * (KP + KH)  # fp16 cols holding the fp32 bias/decay pack

# time-chunk widths; tapered at both ends (see module docstring)
WIDTHS = (128, 256, 512, 512, 512, 512, 512, 512, 384, 128, 128)
# chunks with index >= this store on the sync HWDGE ring instead of SWDGE
SYNC_STORE_FROM = 6
# dummy matmuls at kernel start to warm the PE HAM clock gate during the
# input DMA fill (each ~0.43us cold)
N_WARMUP = 5

_NC_CACHE = {}


def build_nc(widths=WIDTHS):
    key = (widths,)
    if key in _NC_CACHE:
        return _NC_CACHE[key]
    nch = len(widths)
    toffs = [sum(widths[:i]) for i in range(nch)]
    assert sum(widths) == T
    CHMAX = max(widths)
    MOmax = CHMAX // 128

    nc = bacc.Bacc("TRN2", target_bir_lowering=False, debug=False)

    xT_d = nc.dram_tensor("xT", [D, T], F16, kind="ExternalInput")
    # Wi^T k0,k1 pack + fp32 bias/decay pack bit-cast into trailing cols
    wiam_d = nc.dram_tensor("wiam", [128, WIA_F + MP_F], F16, kind="ExternalInput")
    wib_d = nc.dram_tensor("wib", [128, WIA_F], F16, kind="ExternalInput")
    wbb_d = nc.dram_tensor("wbb", [128, KP * H], F16, kind="ExternalInput")
    ww2_d = nc.dram_tensor("ww2", [128, KH * O], F16, kind="ExternalInput")
    out_d = nc.dram_tensor("out", [T, O], F16, kind="ExternalOutput")

    with tile.TileContext(nc) as tc, ExitStack() as ctx:
        wpool = ctx.enter_context(tc.tile_pool(name="weights", bufs=1))
        xpool = ctx.enter_context(tc.tile_pool(name="x", bufs=nch))
        xppool = ctx.enter_context(tc.tile_pool(name="xp", bufs=3))
        hspool = ctx.enter_context(tc.tile_pool(name="hs", bufs=3))
        opool = ctx.enter_context(tc.tile_pool(name="osb", bufs=5))
        psA = ctx.enter_context(tc.tile_pool(name="psA", bufs=2, space="PSUM"))
        psB = ctx.enter_context(tc.tile_pool(name="psB", bufs=2, space="PSUM"))
        psO = ctx.enter_context(tc.tile_pool(name="psO", bufs=4, space="PSUM"))

        xT_v = xT_d.ap().rearrange("(k p) t -> p k t", p=128)
        x_tiles = []

        # ---- startup DMAs: one fused descriptor per ring leg.
        # sync ring: chunk-0 x (narrow chunk -> small descriptor), then the
        # deep x prefetch.
        x0_sb = xpool.tile([128, KD, CHMAX], F16, name="x_sb0", tag="x_sb")
        nc.sync.dma_start(out=x0_sb[:, :, : widths[0]], in_=xT_v[:, :, : widths[0]])
        x_tiles.append(x0_sb)
        for c in range(1, nch):
            w = widths[c]
            x_sb = xpool.tile([128, KD, CHMAX], F16, name=f"x_sb{c}", tag="x_sb")
            nc.sync.dma_start(
                out=x_sb[:, :, :w], in_=xT_v[:, :, toffs[c] : toffs[c] + w]
            )
            x_tiles.append(x_sb)

        # scalar ring: weights in need order, each its own tile+descriptor
        # (LDWEIGHTS' negative-stride reads force whole-tile dependencies)
        wiam_sb = wpool.tile([128, WIA_F + MP_F], F16)
        nc.scalar.dma_start(out=wiam_sb, in_=wiam_d.ap())
        wib_sb = wpool.tile([128, WIA_F], F16)
        nc.scalar.dma_start(out=wib_sb, in_=wib_d.ap())
        wbb_sb = wpool.tile([128, KP * H], F16)
        nc.scalar.dma_start(out=wbb_sb, in_=wbb_d.ap())
        ww2_sb = wpool.tile([128, KH * O], F16)
        nc.scalar.dma_start(out=ww2_sb, in_=ww2_d.ap())

        mview = wiam_sb[:, WIA_F : WIA_F + MP_F].bitcast(F32)  # [128, KP+KH]
        bicol_sl = [mview[:, m : m + 1] for m in range(KP)]
        acol_sl = [mview[:, KP + m : KP + m + 1] for m in range(KH)]

        # ---- PE warmup: dummy matmuls on an unread tile; they run during
        # the DMA fill and trip the HAM clock gate so real matmuls start at
        # 2.4GHz instead of 1.2GHz.
        warm_sb = wpool.tile([128, CHMAX], F16)
        nc.vector.memset(warm_sb[:, :], 0.0)
        for i in range(N_WARMUP):
            warm_ps = psA.tile([128, CHMAX], F32, tag="ps1", name=f"warm_ps{i}")
            nc.tensor.matmul(
                warm_ps[:, :],
                warm_sb[:, :128],
                warm_sb[:, :],
                start=True,
                stop=True,
            )

        def wiT_sl(k, m):  # lhsT tile [128, 128] for MM1
            t_sb = wiam_sb if k < 2 else wib_sb
            kk = k % 2
            return t_sb[:, kk * P + m * 128 : kk * P + (m + 1) * 128]

        def bmat_sl(k, m):
            return wbb_sb[:, k * H + m * 128 : k * H + (m + 1) * 128]

        def w2_sl(k):
            return ww2_sb[:, k * O : (k + 1) * O]

        hs_tiles = [None] * nch

        def mm3_block(c):
            w = widths[c]
            stn = w // 128
            o_sb = opool.tile([128, MOmax, O], F16, name=f"o_sb{c}", tag="o_sb")
            hs_sb = hs_tiles[c]
            out_vc = out_d.ap()[toffs[c] : toffs[c] + w, :].rearrange(
                "(s p) o -> p s o", p=128
            )
            if c == len(widths) - 1:
                st_eng = nc.scalar  # idle ring at the tail -> fast drain
            elif c < SYNC_STORE_FROM:
                st_eng = nc.gpsimd
            else:
                st_eng = nc.sync
            for st in range(stn):
                ps3 = psO.tile([128, O], F32, tag="ps3", name=f"ps3_{c}_{st}")
                for k in range(KH):
                    nc.tensor.matmul(
                        ps3[:, :],
                        hs_sb[:, k, ts(st, 128)],
                        w2_sl(k),
                        start=(k == 0),
                        stop=(k == KH - 1),
                    )
                # alternate DVE/ACT so neither engine saturates (GPSIMD has
                # no PSUM port)
                if st % 2 == 0 and stn > 1:
                    nc.vector.tensor_scalar_add(o_sb[:, st, :], ps3[:, :], 0.0)
                else:
                    nc.scalar.copy(o_sb[:, st, :], ps3[:, :])
            st_eng.dma_start(out=out_vc[:, :stn, :], in_=o_sb[:, :stn, :])

        def mm1_block(c, k_major=False):
            # MM1 + relu/bias -> xp tile; returns the xp tile.  k_major
            # emits k-tile-interleaved matmuls so the first chunks can run
            # on the wiam half while wib's descriptor is still in flight.
            w = widths[c]
            x_sb = x_tiles[c]
            xp_sb = xppool.tile([128, KP, CHMAX], F16, name=f"xp_sb{c}", tag="xp_sb")
            ps1 = [
                psA.tile([128, CHMAX], F32, tag="ps1", name=f"ps1_{c}_{m}")
                for m in range(KP)
            ]
            order = (
                [(k, m) for k in range(KD) for m in range(KP)]
                if k_major
                else [(k, m) for m in range(KP) for k in range(KD)]
            )
            for k, m in order:
                nc.tensor.matmul(
                    ps1[m][:, :w],
                    wiT_sl(k, m),
                    x_sb[:, k, :w],
                    start=(k == 0),
                    stop=(k == KD - 1),
                )
            for m in range(KP):
                nc.scalar.activation(
                    out=xp_sb[:, m, :w],
                    in_=ps1[m][:, :w],
                    func=mybir.ActivationFunctionType.Relu,
                    bias=bicol_sl[m],
                    scale=1.0,
                )
            return xp_sb

        def mm2_scan(c, xp_sb):
            w = widths[c]
            hs_sb = hspool.tile([128, KH, CHMAX], F16, name=f"hs_sb{c}", tag="hs_sb")
            for m in range(KH):
                ps2 = psB.tile([128, CHMAX], F32, tag="ps2", name=f"ps2_{c}_{m}")
                for k in range(KP):
                    nc.tensor.matmul(
                        ps2[:, :w],
                        bmat_sl(k, m),
                        xp_sb[:, k, :w],
                        start=(k == 0),
                        stop=(k == KP - 1),
                    )
                init = (
                    0.0
                    if c == 0
                    else hs_tiles[c - 1][:, m, widths[c - 1] - 1 : widths[c - 1]]
                )
                nc.vector.tensor_tensor_scan(
                    out=hs_sb[:, m, :w],
                    data0=acol_sl[m].broadcast_to((128, w)),
                    data1=ps2[:, :w],
                    initial=init,
                    op0=mybir.AluOpType.mult,
                    op1=mybir.AluOpType.add,
                )
            hs_tiles[c] = hs_sb

        # chunks 0,1 are software-pipelined ahead of MM2 so the PE has
        # runway while the wib/wbb descriptors are still in flight
        xp0 = mm1_block(0, k_major=True)
        xp1 = mm1_block(1, k_major=True)
        mm2_scan(0, xp0)
        mm2_scan(1, xp1)
        mm3_block(0)
        for c in range(2, nch):
            xp_sb = mm1_block(c)
            mm2_scan(c, xp_sb)
            # deferred MM3 of the previous chunk (keeps PE off the scan's
            # critical path)
            mm3_block(c - 1)
        mm3_block(nch - 1)

    nc.finalize()
    _NC_CACHE[key] = nc
    return nc


def _pack128(w, kt):  # [kt*128, F] -> [128, kt*F]
    return np.transpose(w.reshape(kt, 128, -1), (1, 0, 2)).reshape(128, -1)


def _prep_shared(a, B, C, Wi, bi, Wo, bo):
    w2 = (C.astype(np.float64) @ Wo.astype(np.float64).T).astype(np.float32)
    wiT = np.ascontiguousarray(Wi.T)
    wi16 = _pack128(wiT, KD).astype(np.float16)  # [128, KD*P]
    mp32 = np.ascontiguousarray(
        np.concatenate([bi.reshape(KP, 128).T, a.reshape(KH, 128).T], axis=1)
    ).astype(np.float32)  # [128, KP+KH]
    mp16 = mp32.view(np.float16)  # same bytes as [128, 2*(KP+KH)] fp16
    return {
        "wiam": np.ascontiguousarray(
            np.concatenate([wi16[:, : 2 * P], mp16], axis=1)
        ),
        "wib": np.ascontiguousarray(wi16[:, 2 * P :]),
        "wbb": np.ascontiguousarray(_pack128(B, KP).astype(np.float16)),
        "ww2": np.ascontiguousarray(_pack128(w2, KH).astype(np.float16)),
    }


def kernel(x_seq, a, B, C, Wi, bi, Wo, bo, _collect=None):
    nc = build_nc()
    shared = _prep_shared(a, B, C, Wi, bi, Wo, bo)
    in_maps = []
    for b in range(BSZ):
        m = dict(shared)
        m["xT"] = np.ascontiguousarray(x_seq[b].T.astype(np.float16))
        in_maps.append(m)
    kwargs = {}
    if _collect is not None:
        kwargs = {k: v for k, v in _collect.items() if k != "res"}
    try:
        res = run_bass_kernel_spmd(nc, in_maps, core_ids=list(range(BSZ)), **kwargs)
    except Exception:
        # one retry for transient device errors
        res = run_bass_kernel_spmd(nc, in_maps, core_ids=list(range(BSZ)), **kwargs)
    if _collect is not None:
        _collect["res"] = res
    out = np.stack([res.results[b]["out"] for b in range(BSZ)], axis=0)
    # device output is biasless fp16; apply +bo exactly in fp32 on host
    return out.astype(np.float32) + bo[None, None, :].astype(np.float32)


# revision 21
# speedup vs baseline: 1.0581x; 1.0581x over previous
"""Trainium2 Bass kernel for DiscreteTimeS4.

Reference computation (per batch element b):
    x_proj = relu(x @ Wi^T + bi)          [T, P]
    u      = x_proj @ B                   [T, H]
    h_t    = a * h_{t-1} + u_t            (diagonal linear scan over T)
    y      = hs @ C                       [T, P]
    out    = y @ Wo^T + bo                [T, O]

Sharding: data-parallel over the batch axis -- core b handles x_seq[b].
Weights replicated. No cross-device communication.

Device strategy (per core):
  - host pre-transposes x to x^T [D, T] so the contraction dim (D) sits on
    SBUF partitions; host fuses W2 = C @ Wo^T so the last two matmuls
    collapse into one; host applies the +bo bias (exact in fp32) so the
    device ships biasless fp16 output (half the store traffic).
  - matmuls run in fp16 (fp32 PSUM accumulation, FWL weight loads); the
    scan runs in exact fp32.
  - startup: DMA descriptors have ~1.3-2us serial per-ring latency, so the
    critical path uses FEW, FUSED descriptors on parallel rings: sync ring
    carries x chunk 0 (one descriptor -- chunk 0 is narrow) then the deep
    x prefetch; the scalar ring carries ALL MM1 weights + the fp32
    bias/decay pack bit-packed into trailing fp16 columns (descriptor 1)
    and the fused B|W2 pack (descriptor 2).  A short stream of dummy
    warmup matmuls runs during the DMA fill to trip the PE HAM clock gate
    (cold 1.2GHz -> warm 2.4GHz) before real work.
  - pipeline over time chunks, tapered on both ends (small first chunk =
    small critical-path DMA; small last chunks = short post-scan tail):
      MM1: XP^T[p,t] = Wi @ x^T          (lhsT = Wi^T, K=512)
      ACT: relu(psum + bi) -> SBUF
      MM2: U^T[h,t] = B^T @ XP^T         (lhsT = B, K=256)
      DVE: tensor_tensor_scan: h = a*h + u along t (fp32 state, carry
           chained across chunks via initial=prev[:, -1:], decay broadcast
           via a zero-stride AP)
      MM3: OUT[t,o] = (HS^T)^T @ W2      (lhsT = HS^T tile -> natural [t,o]
           output layout)
      psum->fp16 evacuation alternates DVE/ACT per 128-row subtile so
      neither engine saturates (GPSIMD has no PSUM port), then one DMA per
      chunk: early chunks on the SWDGE ring, late chunks on sync HWDGE.
"""

import numpy as np

try:
    import concourse.bass as bass
except ImportError:  # pragma: no cover
    import sys

    sys.path.insert(0, "/opt/trn_rl_repo")
    import concourse.bass as bass

from contextlib import ExitStack

import concourse.mybir as mybir
import concourse.tile as tile
from concourse import bacc
from concourse.bass import ts
from concourse.bass_utils import run_bass_kernel_spmd

BSZ, T, D, P, H, O = 8, 4096, 512, 256, 256, 512
F32 = mybir.dt.float32
F16 = mybir.dt.float16

KD = D // 128  # 4 k-tiles for MM1
KP = P // 128  # 2
KH = H // 128  # 2

WI_F = KD * P  # fp16 cols of packed Wi^T
MP_F = 2 * (KP + KH)  # fp16 cols holding the fp32 bias/decay pack

# time-chunk widths; tapered at both ends (see module docstring)
WIDTHS = (256, 256, 512, 512, 512, 512, 512, 512, 256, 128, 128)
# chunks with index >= this store on the sync HWDGE ring instead of SWDGE
SYNC_STORE_FROM = 6
# dummy matmuls at kernel start to warm the PE HAM clock gate during the
# input DMA fill; sized to end right when the weight descriptor lands
N_WARMUP = 8

_NC_CACHE = {}


def build_nc(widths=WIDTHS):
    key = (widths,)
    if key in _NC_CACHE:
        return _NC_CACHE[key]
    nch = len(widths)
    toffs = [sum(widths[:i]) for i in range(nch)]
    assert sum(widths) == T
    CHMAX = max(widths)
    MOmax = CHMAX // 128

    nc = bacc.Bacc("TRN2", target_bir_lowering=False, debug=False)

    xT_d = nc.dram_tensor("xT", [D, T], F16, kind="ExternalInput")
    # Wi^T pack + fp32 bias/decay pack bit-cast into trailing fp16 columns
    wim_d = nc.dram_tensor("wim", [128, WI_F + MP_F], F16, kind="ExternalInput")
    # B pack | W2 pack fused
    wb16_d = nc.dram_tensor("wb16", [128, KP * H + KH * O], F16, kind="ExternalInput")
    out_d = nc.dram_tensor("out", [T, O], F16, kind="ExternalOutput")

    with tile.TileContext(nc) as tc, ExitStack() as ctx:
        wpool = ctx.enter_context(tc.tile_pool(name="weights", bufs=1))
        xpool = ctx.enter_context(tc.tile_pool(name="x", bufs=nch))
        xppool = ctx.enter_context(tc.tile_pool(name="xp", bufs=3))
        hspool = ctx.enter_context(tc.tile_pool(name="hs", bufs=3))
        opool = ctx.enter_context(tc.tile_pool(name="osb", bufs=5))
        psA = ctx.enter_context(tc.tile_pool(name="psA", bufs=2, space="PSUM"))
        psB = ctx.enter_context(tc.tile_pool(name="psB", bufs=2, space="PSUM"))
        psO = ctx.enter_context(tc.tile_pool(name="psO", bufs=4, space="PSUM"))

        xT_v = xT_d.ap().rearrange("(k p) t -> p k t", p=128)
        x_tiles = []

        # ---- startup DMAs: one fused descriptor per ring leg.
        # sync ring: chunk-0 x (narrow chunk -> small descriptor), then the
        # deep x prefetch.
        x0_sb = xpool.tile([128, KD, CHMAX], F16, name="x_sb0", tag="x_sb")
        nc.sync.dma_start(out=x0_sb[:, :, : widths[0]], in_=xT_v[:, :, : widths[0]])
        x_tiles.append(x0_sb)
        for c in range(1, nch):
            w = widths[c]
            x_sb = xpool.tile([128, KD, CHMAX], F16, name=f"x_sb{c}", tag="x_sb")
            nc.sync.dma_start(
                out=x_sb[:, :, :w], in_=xT_v[:, :, toffs[c] : toffs[c] + w]
            )
            x_tiles.append(x_sb)

        # scalar ring: all MM1 weights + bias/decay pack (one descriptor),
        # then B|W2 (one descriptor).  Descriptors have ~1.3-2us serial
        # per-ring latency, so fewer + fused wins over fine splitting.
        wim_sb = wpool.tile([128, WI_F + MP_F], F16)
        nc.scalar.dma_start(out=wim_sb, in_=wim_d.ap())
        wb16_sb = wpool.tile([128, KP * H + KH * O], F16)
        nc.scalar.dma_start(out=wb16_sb, in_=wb16_d.ap())

        mview = wim_sb[:, WI_F : WI_F + MP_F].bitcast(F32)  # [128, KP+KH]
        bicol_sl = [mview[:, m : m + 1] for m in range(KP)]
        acol_sl = [mview[:, KP + m : KP + m + 1] for m in range(KH)]

        # ---- PE warmup: dummy matmuls on an unread tile; they run during
        # the DMA fill and trip the HAM clock gate so real matmuls start at
        # 2.4GHz instead of 1.2GHz.
        warm_sb = wpool.tile([128, CHMAX], F16)
        nc.vector.memset(warm_sb[:, :], 0.0)
        for i in range(N_WARMUP):
            warm_ps = psA.tile([128, CHMAX], F32, tag="ps1", name=f"warm_ps{i}")
            nc.tensor.matmul(
                warm_ps[:, :],
                warm_sb[:, :128],
                warm_sb[:, :],
                start=True,
                stop=True,
            )

        def wiT_sl(k, m):  # lhsT tile [128, 128] for MM1
            return wim_sb[:, k * P + m * 128 : k * P + (m + 1) * 128]

        def bmat_sl(k, m):
            return wb16_sb[:, k * H + m * 128 : k * H + (m + 1) * 128]

        def w2_sl(k):
            return wb16_sb[:, KP * H + k * O : KP * H + (k + 1) * O]

        hs_tiles = [None] * nch

        def mm3_block(c):
            w = widths[c]
            stn = w // 128
            o_sb = opool.tile([128, MOmax, O], F16, name=f"o_sb{c}", tag="o_sb")
            hs_sb = hs_tiles[c]
            out_vc = out_d.ap()[toffs[c] : toffs[c] + w, :].rearrange(
                "(s p) o -> p s o", p=128
            )
            st_eng = nc.gpsimd if c < SYNC_STORE_FROM else nc.sync
            for st in range(stn):
                ps3 = psO.tile([128, O], F32, tag="ps3", name=f"ps3_{c}_{st}")
                for k in range(KH):
                    nc.tensor.matmul(
                        ps3[:, :],
                        hs_sb[:, k, ts(st, 128)],
                        w2_sl(k),
                        start=(k == 0),
                        stop=(k == KH - 1),
                    )
                # alternate DVE/ACT so neither engine saturates (GPSIMD has
                # no PSUM port); the final chunks get the lowest-latency
                # evacuation since they sit on the exit critical path
                if c == len(widths) - 1:
                    # split halves across both engines in parallel
                    nc.vector.tensor_scalar_add(
                        o_sb[:, st, : O // 2], ps3[:, : O // 2], 0.0
                    )
                    nc.scalar.copy(o_sb[:, st, O // 2 :], ps3[:, O // 2 :])
                elif stn == 1:
                    nc.vector.tensor_scalar_add(o_sb[:, st, :], ps3[:, :], 0.0)
                elif st % 2 == 0:
                    nc.vector.tensor_scalar_add(o_sb[:, st, :], ps3[:, :], 0.0)
                else:
                    nc.scalar.copy(o_sb[:, st, :], ps3[:, :])
            st_eng.dma_start(out=out_vc[:, :stn, :], in_=o_sb[:, :stn, :])

        def mm1_block(c, k_major=False):
            # MM1 + relu/bias -> xp tile; returns the xp tile.  k_major
            # emits k-tile-interleaved matmuls so the first chunks can run
            # on the wiam half while wib's descriptor is still in flight.
            w = widths[c]
            x_sb = x_tiles[c]
            xp_sb = xppool.tile([128, KP, CHMAX], F16, name=f"xp_sb{c}", tag="xp_sb")
            ps1 = [
                psA.tile([128, CHMAX], F32, tag="ps1", name=f"ps1_{c}_{m}")
                for m in range(KP)
            ]
            order = (
                [(k, m) for k in range(KD) for m in range(KP)]
                if k_major
                else [(k, m) for m in range(KP) for k in range(KD)]
            )
            for k, m in order:
                nc.tensor.matmul(
                    ps1[m][:, :w],
                    wiT_sl(k, m),
                    x_sb[:, k, :w],
                    start=(k == 0),
                    stop=(k == KD - 1),
                )
            for m in range(KP):
                nc.scalar.activation(
                    out=xp_sb[:, m, :w],
                    in_=ps1[m][:, :w],
                    func=mybir.ActivationFunctionType.Relu,
                    bias=bicol_sl[m],
                    scale=1.0,
                )
            return xp_sb

        def mm2_scan(c, xp_sb):
            w = widths[c]
            hs_sb = hspool.tile([128, KH, CHMAX], F16, name=f"hs_sb{c}", tag="hs_sb")
            for m in range(KH):
                ps2 = psB.tile([128, CHMAX], F32, tag="ps2", name=f"ps2_{c}_{m}")
                for k in range(KP):
                    nc.tensor.matmul(
                        ps2[:, :w],
                        bmat_sl(k, m),
                        xp_sb[:, k, :w],
                        start=(k == 0),
                        stop=(k == KP - 1),
                    )
                init = (
                    0.0
                    if c == 0
                    else hs_tiles[c - 1][:, m, widths[c - 1] - 1 : widths[c - 1]]
                )
                nc.vector.tensor_tensor_scan(
                    out=hs_sb[:, m, :w],
                    data0=acol_sl[m].broadcast_to((128, w)),
                    data1=ps2[:, :w],
                    initial=init,
                    op0=mybir.AluOpType.mult,
                    op1=mybir.AluOpType.add,
                )
            hs_tiles[c] = hs_sb

        # chunks 0,1 are software-pipelined ahead of MM2 so the PE has
        # runway while the wib/wbb descriptors are still in flight
        xp0 = mm1_block(0, k_major=True)
        xp1 = mm1_block(1, k_major=True)
        mm2_scan(0, xp0)
        mm2_scan(1, xp1)
        mm3_block(0)
        for c in range(2, nch):
            xp_sb = mm1_block(c)
            mm2_scan(c, xp_sb)
            # deferred MM3 of the previous chunk (keeps PE off the scan's
            # critical path)
            mm3_block(c - 1)
        mm3_block(nch - 1)

    nc.finalize()
    _NC_CACHE[key] = nc
    return nc


def _pack128(w, kt):  # [kt*128, F] -> [128, kt*F]
    return np.transpose(w.reshape(kt, 128, -1), (1, 0, 2)).reshape(128, -1)


def _prep_shared(a, B, C, Wi, bi, Wo, bo):
    w2 = (C.astype(np.float64) @ Wo.astype(np.float64).T).astype(np.float32)
    wiT = np.ascontiguousarray(Wi.T)
    wi16 = _pack128(wiT, KD).astype(np.float16)  # [128, KD*P]
    mp32 = np.ascontiguousarray(
        np.concatenate([bi.reshape(KP, 128).T, a.reshape(KH, 128).T], axis=1)
    ).astype(np.float32)  # [128, KP+KH]
    mp16 = mp32.view(np.float16)  # same bytes as [128, 2*(KP+KH)] fp16
    return {
        "wim": np.ascontiguousarray(np.concatenate([wi16, mp16], axis=1)),
        "wb16": np.ascontiguousarray(
            np.concatenate([_pack128(B, KP), _pack128(w2, KH)], axis=1).astype(
                np.float16
            )
        ),
    }


def kernel(x_seq, a, B, C, Wi, bi, Wo, bo, _collect=None):
    nc = build_nc()
    shared = _prep_shared(a, B, C, Wi, bi, Wo, bo)
    in_maps = []
    for b in range(BSZ):
        m = dict(shared)
        m["xT"] = np.ascontiguousarray(x_seq[b].T.astype(np.float16))
        in_maps.append(m)
    kwargs = {}
    if _collect is not None:
        kwargs = {k: v for k, v in _collect.items() if k != "res"}
    try:
        res = run_bass_kernel_spmd(nc, in_maps, core_ids=list(range(BSZ)), **kwargs)
    except Exception:
        # one retry for transient device errors
        res = run_bass_kernel_spmd(nc, in_maps, core_ids=list(range(BSZ)), **kwargs)
    if _collect is not None:
        _collect["res"] = res
    out = np.stack([res.results[b]["out"] for b in range(BSZ)], axis=0)
    # device output is biasless fp16; apply +bo exactly in fp32 on host
    return out.astype(np.float32) + bo[None, None, :].astype(np.float32)


# revision 22
# speedup vs baseline: 1.0916x; 1.0317x over previous
"""Trainium2 Bass kernel for DiscreteTimeS4.

Reference computation (per batch element b):
    x_proj = relu(x @ Wi^T + bi)          [T, P]
    u      = x_proj @ B                   [T, H]
    h_t    = a * h_{t-1} + u_t            (diagonal linear scan over T)
    y      = hs @ C                       [T, P]
    out    = y @ Wo^T + bo                [T, O]

Sharding: data-parallel over the batch axis -- core b handles x_seq[b].
Weights replicated. No cross-device communication.

Device strategy (per core):
  - host pre-transposes x to x^T [D, T] so the contraction dim (D) sits on
    SBUF partitions; host fuses W2 = C @ Wo^T so the last two matmuls
    collapse into one; host applies the +bo bias (exact in fp32) so the
    device ships biasless fp16 output (half the store traffic).
  - matmuls run in fp16 (fp32 PSUM accumulation, FWL weight loads); the
    scan runs in exact fp32.
  - startup: DMA descriptors have ~1.3-2us serial per-ring latency, so the
    critical path uses FEW, FUSED descriptors on parallel rings: sync ring
    carries x chunk 0 (one descriptor -- chunk 0 is narrow) then the deep
    x prefetch; the scalar ring carries ALL MM1 weights + the fp32
    bias/decay pack bit-packed into trailing fp16 columns (descriptor 1)
    and the fused B|W2 pack (descriptor 2).  A short stream of dummy
    warmup matmuls runs during the DMA fill to trip the PE HAM clock gate
    (cold 1.2GHz -> warm 2.4GHz) before real work.
  - pipeline over time chunks, tapered on both ends (small first chunk =
    small critical-path DMA; small last chunks = short post-scan tail):
      MM1: XP^T[p,t] = Wi @ x^T          (lhsT = Wi^T, K=512)
      ACT: relu(psum + bi) -> SBUF
      MM2: U^T[h,t] = B^T @ XP^T         (lhsT = B, K=256)
      DVE: tensor_tensor_scan: h = a*h + u along t (fp32 state, carry
           chained across chunks via initial=prev[:, -1:], decay broadcast
           via a zero-stride AP)
      MM3: OUT[t,o] = (HS^T)^T @ W2      (lhsT = HS^T tile -> natural [t,o]
           output layout)
      psum->fp16 evacuation alternates DVE/ACT per 128-row subtile so
      neither engine saturates (GPSIMD has no PSUM port), then one DMA per
      chunk: early chunks on the SWDGE ring, late chunks on sync HWDGE.
"""

import numpy as np

try:
    import concourse.bass as bass
except ImportError:  # pragma: no cover
    import sys

    sys.path.insert(0, "/opt/trn_rl_repo")
    import concourse.bass as bass

from contextlib import ExitStack

import concourse.mybir as mybir
import concourse.tile as tile
from concourse import bacc
from concourse.bass import ts
from concourse.bass_utils import run_bass_kernel_spmd

BSZ, T, D, P, H, O = 8, 4096, 512, 256, 256, 512
F32 = mybir.dt.float32
F16 = mybir.dt.float16

KD = D // 128  # 4 k-tiles for MM1
KP = P // 128  # 2
KH = H // 128  # 2

WI_F = KD * P  # fp16 cols of packed Wi^T
MP_F = 2 * (KP + KH)  # fp16 cols holding the fp32 bias/decay pack

# time-chunk widths; tapered at both ends (see module docstring)
WIDTHS = (256, 256, 512, 512, 512, 512, 512, 512, 256, 128, 128)
# chunks with index >= this store on the sync HWDGE ring instead of SWDGE
SYNC_STORE_FROM = 6
# dummy matmuls at kernel start to warm the PE HAM clock gate during the
# input DMA fill; sized to end right when the weight descriptor lands
N_WARMUP = 8

_NC_CACHE = {}


def build_nc(widths=WIDTHS):
    key = (widths,)
    if key in _NC_CACHE:
        return _NC_CACHE[key]
    nch = len(widths)
    toffs = [sum(widths[:i]) for i in range(nch)]
    assert sum(widths) == T
    CHMAX = max(widths)
    MOmax = CHMAX // 128

    nc = bacc.Bacc("TRN2", target_bir_lowering=False, debug=False)

    xT_d = nc.dram_tensor("xT", [D, T], F16, kind="ExternalInput")
    # Wi^T pack + fp32 bias/decay pack bit-cast into trailing fp16 columns
    wim_d = nc.dram_tensor("wim", [128, WI_F + MP_F], F16, kind="ExternalInput")
    # B pack | W2 pack fused
    wb16_d = nc.dram_tensor("wb16", [128, KP * H + KH * O], F16, kind="ExternalInput")
    out_d = nc.dram_tensor("out", [T, O], F16, kind="ExternalOutput")

    with tile.TileContext(nc) as tc, ExitStack() as ctx:
        wpool = ctx.enter_context(tc.tile_pool(name="weights", bufs=1))
        xpool = ctx.enter_context(tc.tile_pool(name="x", bufs=nch))
        xppool = ctx.enter_context(tc.tile_pool(name="xp", bufs=3))
        hspool = ctx.enter_context(tc.tile_pool(name="hs", bufs=3))
        opool = ctx.enter_context(tc.tile_pool(name="osb", bufs=5))
        psA = ctx.enter_context(tc.tile_pool(name="psA", bufs=2, space="PSUM"))
        psB = ctx.enter_context(tc.tile_pool(name="psB", bufs=2, space="PSUM"))
        psO = ctx.enter_context(tc.tile_pool(name="psO", bufs=4, space="PSUM"))

        xT_v = xT_d.ap().rearrange("(k p) t -> p k t", p=128)
        x_tiles = []

        # ---- startup DMAs: one fused descriptor per ring leg.
        # sync ring: chunk-0 x (narrow chunk -> small descriptor), then the
        # deep x prefetch.
        x0_sb = xpool.tile([128, KD, CHMAX], F16, name="x_sb0", tag="x_sb")
        nc.sync.dma_start(out=x0_sb[:, :, : widths[0]], in_=xT_v[:, :, : widths[0]])
        x_tiles.append(x0_sb)
        for c in range(1, nch):
            w = widths[c]
            x_sb = xpool.tile([128, KD, CHMAX], F16, name=f"x_sb{c}", tag="x_sb")
            nc.sync.dma_start(
                out=x_sb[:, :, :w], in_=xT_v[:, :, toffs[c] : toffs[c] + w]
            )
            x_tiles.append(x_sb)

        # scalar ring: all MM1 weights + bias/decay pack (one descriptor),
        # then B|W2 (one descriptor).  Descriptors have ~1.3-2us serial
        # per-ring latency, so fewer + fused wins over fine splitting.
        wim_sb = wpool.tile([128, WI_F + MP_F], F16)
        nc.scalar.dma_start(out=wim_sb, in_=wim_d.ap())
        wb16_sb = wpool.tile([128, KP * H + KH * O], F16)
        nc.scalar.dma_start(out=wb16_sb, in_=wb16_d.ap())

        mview = wim_sb[:, WI_F : WI_F + MP_F].bitcast(F32)  # [128, KP+KH]
        bicol_sl = [mview[:, m : m + 1] for m in range(KP)]
        acol_sl = [mview[:, KP + m : KP + m + 1] for m in range(KH)]

        # ---- PE warmup: dummy matmuls on an unread tile; they run during
        # the DMA fill and trip the HAM clock gate so real matmuls start at
        # 2.4GHz instead of 1.2GHz.
        warm_sb = wpool.tile([128, CHMAX], F16)
        nc.vector.memset(warm_sb[:, :], 0.0)
        for i in range(N_WARMUP):
            warm_ps = psA.tile([128, CHMAX], F32, tag="ps1", name=f"warm_ps{i}")
            nc.tensor.matmul(
                warm_ps[:, :],
                warm_sb[:, :128],
                warm_sb[:, :],
                start=True,
                stop=True,
            )

        def wiT_sl(k, m):  # lhsT tile [128, 128] for MM1
            return wim_sb[:, k * P + m * 128 : k * P + (m + 1) * 128]

        def bmat_sl(k, m):
            return wb16_sb[:, k * H + m * 128 : k * H + (m + 1) * 128]

        def w2_sl(k):
            return wb16_sb[:, KP * H + k * O : KP * H + (k + 1) * O]

        hs_tiles = [None] * nch

        def mm3_block(c):
            w = widths[c]
            stn = w // 128
            o_sb = opool.tile([128, MOmax, O], F16, name=f"o_sb{c}", tag="o_sb")
            hs_sb = hs_tiles[c]
            out_vc = out_d.ap()[toffs[c] : toffs[c] + w, :].rearrange(
                "(s p) o -> p s o", p=128
            )
            st_eng = nc.gpsimd if c < SYNC_STORE_FROM else nc.sync
            for st in range(stn):
                ps3 = psO.tile([128, O], F32, tag="ps3", name=f"ps3_{c}_{st}")
                for k in range(KH):
                    nc.tensor.matmul(
                        ps3[:, :],
                        hs_sb[:, k, ts(st, 128)],
                        w2_sl(k),
                        start=(k == 0),
                        stop=(k == KH - 1),
                    )
                # alternate DVE/ACT so neither engine saturates (GPSIMD has
                # no PSUM port).  The last chunk evacuates on DVE, which is
                # guaranteed idle right after the final scan; the
                # second-to-last goes to ACT so the two overlap.
                if c == len(widths) - 1:
                    nc.vector.tensor_scalar_add(o_sb[:, st, :], ps3[:, :], 0.0)
                elif stn == 1:
                    nc.scalar.copy(o_sb[:, st, :], ps3[:, :])
                elif st % 2 == 0:
                    nc.vector.tensor_scalar_add(o_sb[:, st, :], ps3[:, :], 0.0)
                else:
                    nc.scalar.copy(o_sb[:, st, :], ps3[:, :])
            st_eng.dma_start(out=out_vc[:, :stn, :], in_=o_sb[:, :stn, :])

        def mm1_block(c, k_major=False):
            # MM1 + relu/bias -> xp tile; returns the xp tile.  k_major
            # emits k-tile-interleaved matmuls so the first chunks can run
            # on the wiam half while wib's descriptor is still in flight.
            w = widths[c]
            x_sb = x_tiles[c]
            xp_sb = xppool.tile([128, KP, CHMAX], F16, name=f"xp_sb{c}", tag="xp_sb")
            ps1 = [
                psA.tile([128, CHMAX], F32, tag="ps1", name=f"ps1_{c}_{m}")
                for m in range(KP)
            ]
            order = (
                [(k, m) for k in range(KD) for m in range(KP)]
                if k_major
                else [(k, m) for m in range(KP) for k in range(KD)]
            )
            for k, m in order:
                nc.tensor.matmul(
                    ps1[m][:, :w],
                    wiT_sl(k, m),
                    x_sb[:, k, :w],
                    start=(k == 0),
                    stop=(k == KD - 1),
                )
            for m in range(KP):
                nc.scalar.activation(
                    out=xp_sb[:, m, :w],
                    in_=ps1[m][:, :w],
                    func=mybir.ActivationFunctionType.Relu,
                    bias=bicol_sl[m],
                    scale=1.0,
                )
            return xp_sb

        def mm2_scan(c, xp_sb):
            w = widths[c]
            hs_sb = hspool.tile([128, KH, CHMAX], F16, name=f"hs_sb{c}", tag="hs_sb")
            for m in range(KH):
                ps2 = psB.tile([128, CHMAX], F32, tag="ps2", name=f"ps2_{c}_{m}")
                for k in range(KP):
                    nc.tensor.matmul(
                        ps2[:, :w],
                        bmat_sl(k, m),
                        xp_sb[:, k, :w],
                        start=(k == 0),
                        stop=(k == KP - 1),
                    )
                init = (
                    0.0
                    if c == 0
                    else hs_tiles[c - 1][:, m, widths[c - 1] - 1 : widths[c - 1]]
                )
                nc.vector.tensor_tensor_scan(
                    out=hs_sb[:, m, :w],
                    data0=acol_sl[m].broadcast_to((128, w)),
                    data1=ps2[:, :w],
                    initial=init,
                    op0=mybir.AluOpType.mult,
                    op1=mybir.AluOpType.add,
                )
            hs_tiles[c] = hs_sb

        # chunks 0,1 are software-pipelined ahead of MM2 so the PE has
        # runway while the wib/wbb descriptors are still in flight
        xp0 = mm1_block(0, k_major=True)
        xp1 = mm1_block(1, k_major=True)
        mm2_scan(0, xp0)
        mm2_scan(1, xp1)
        mm3_block(0)
        for c in range(2, nch):
            xp_sb = mm1_block(c)
            mm2_scan(c, xp_sb)
            # deferred MM3 of the previous chunk (keeps PE off the scan's
            # critical path)
            mm3_block(c - 1)
        mm3_block(nch - 1)

    nc.finalize()
    _NC_CACHE[key] = nc
    return nc


def _pack128(w, kt):  # [kt*128, F] -> [128, kt*F]
    return np.transpose(w.reshape(kt, 128, -1), (1, 0, 2)).reshape(128, -1)


def _prep_shared(a, B, C, Wi, bi, Wo, bo):
    w2 = (C.astype(np.float64) @ Wo.astype(np.float64).T).astype(np.float32)
    wiT = np.ascontiguousarray(Wi.T)
    wi16 = _pack128(wiT, KD).astype(np.float16)  # [128, KD*P]
    mp32 = np.ascontiguousarray(
        np.concatenate([bi.reshape(KP, 128).T, a.reshape(KH, 128).T], axis=1)
    ).astype(np.float32)  # [128, KP+KH]
    mp16 = mp32.view(np.float16)  # same bytes as [128, 2*(KP+KH)] fp16
    return {
        "wim": np.ascontiguousarray(np.concatenate([wi16, mp16], axis=1)),
        "wb16": np.ascontiguousarray(
            np.concatenate([_pack128(B, KP), _pack128(w2, KH)], axis=1).astype(
                np.float16
            )
        ),
    }


def kernel(x_seq, a, B, C, Wi, bi, Wo, bo, _collect=None):
    nc = build_nc()
    shared = _prep_shared(a, B, C, Wi, bi, Wo, bo)
    in_maps = []
    for b in range(BSZ):
        m = dict(shared)
        m["xT"] = np.ascontiguousarray(x_seq[b].T.astype(np.float16))
        in_maps.append(m)
    kwargs = {}
    if _collect is not None:
        kwargs = {k: v for k, v in _collect.items() if k != "res"}
    try:
        res = run_bass_kernel_spmd(nc, in_maps, core_ids=list(range(BSZ)), **kwargs)
    except Exception:
        # one retry for transient device errors
        res = run_bass_kernel_spmd(nc, in_maps, core_ids=list(range(BSZ)), **kwargs)
    if _collect is not None:
        _collect["res"] = res
    out = np.stack([res.results[b]["out"] for b in range(BSZ)], axis=0)
    # device output is biasless fp16; apply +bo exactly in fp32 on host
    return out.astype(np.float32) + bo[None, None, :].astype(np.float32)
